# revision 34
# baseline (speedup 1.0000x reference)
"""Trainium2 Bass kernel: multi-head attention (B,C,S,H)=(2,4,1024,2048), NH=16, HD=128.

Strategy: pure data-parallel over the 8 B*C batch elements -> 8 NeuronCores,
no collectives.  Per core, 3 phases (power-aware: the chip P0-downclocks to
~2.0 GHz under sustained high engine activity, so the schedule keeps total
non-PE engine work low rather than maximally dense):
  phase A: Q^T,K^T projection (transposed [head-dim, seq] layout) with RoPE
           fused into the PSUM eviction (bf16 arithmetic where possible).
           The first d-tile runs kt-major so matmuls chase the xT DMA stream.
  phase A2: V projection (natural [seq, head-dim] layout).
  phase B: per-head attention: scores^T -> ScalarE exp -> AV matmul; softmax
           denominator (DVE tree-sum + ones-matmul broadcast + reciprocal)
           optionally pipelined one head back so the PE never waits on it.
  phase C: O-projection from resident attn^T tiles, streamed w_o.
All matmuls bf16; f32 accumulation in PSUM.  Host pre-transposes/pre-tiles/
casts inputs so every DMA is partition-major contiguous.
"""

import numpy as np
import ml_dtypes

try:
    import concourse  # noqa: F401
except ImportError:
    import sys
    sys.path.insert(0, "/opt/trn_rl_repo")

BF = ml_dtypes.bfloat16

B, C, S, H = 2, 4, 1024, 2048
NH, HD, NENC = 16, 128, 1008
NCORES = 8
KT = H // 128          # 16 contraction tiles for the projections
DT = H // 128          # 16 output d-tiles (heads) for Q/K
ST = S // 128          # 8 seq tiles
SCHUNK = 512
NSC = S // SCHUNK      # 2 seq chunks
SCALE = 1.0 / float(np.sqrt(HD))

ROPE_BF16 = True       # bf16 rope mul/add (less DVE activity)
KT_MAJOR_START = True  # first Q d-tile kt-major to chase the xT DMA stream
PIPELINE_DEN = True    # issue denominator of head h-1 inside head h's loop


def build_nc():
    import concourse.bass as bass
    import concourse.mybir as mybir
    import concourse.tile as tile
    from concourse import bacc

    f32 = mybir.dt.float32
    bf16 = mybir.dt.bfloat16
    ropedt = bf16 if ROPE_BF16 else f32

    nc = bacc.Bacc(None, target_bir_lowering=False, debug=False)

    xT = nc.dram_tensor("xT", [128, KT * S], bf16, kind="ExternalInput")
    wq = nc.dram_tensor("wq", [128, DT * H], bf16, kind="ExternalInput")
    wk = nc.dram_tensor("wk", [128, DT * H], bf16, kind="ExternalInput")
    wv = nc.dram_tensor("wv", [128, 4 * KT * 512], bf16, kind="ExternalInput")
    wo = nc.dram_tensor("wo", [128, DT * H], bf16, kind="ExternalInput")
    cosT = nc.dram_tensor("cosT", [128, S], f32, kind="ExternalInput")
    sinTs = nc.dram_tensor("sinTs", [128, S], f32, kind="ExternalInput")
    out = nc.dram_tensor("out", [H, S], f32, kind="ExternalOutput")

    with tile.TileContext(nc) as tc:
        import contextlib
        with contextlib.ExitStack() as ctx:
            # ---- persistent SBUF tiles -------------------------------------
            persist = ctx.enter_context(tc.tile_pool(name="persist", bufs=1))
            qT_sb = persist.tile([128, NH * S], bf16, tag="qT")
            kT_sb = persist.tile([128, NH * S], bf16, tag="kT")
            v_sb = persist.tile([128, ST * H], bf16, tag="v")

            # ---- phase A: Q^T / K^T projection with fused RoPE -------------
            with tc.tile_pool(name="xpool", bufs=1) as x_pool, \
                 tc.tile_pool(name="trig", bufs=1) as trig_pool, \
                 tc.tile_pool(name="wqk_stream", bufs=3) as wqk_pool, \
                 tc.tile_pool(name="rope_scratch", bufs=2) as rope_pool, \
                 tc.tile_pool(name="psumA", bufs=4, space="PSUM") as psA:

                cos_sb = trig_pool.tile([128, S], f32, tag="cos")
                sin_sb = trig_pool.tile([128, S], ropedt, tag="sin")
                # xT as 16 per-kt tiles: Tile tracks dependencies per tile, so
                # matmuls gate on their own k-tile's DMA, not the full 4.2MB
                xt = [x_pool.tile([128, S], bf16, tag=f"xT{kt}", name=f"xT{kt}")
                      for kt in range(KT)]
                wt_pre = {}
                for dt in range(3):
                    wt_pre[dt] = wqk_pool.tile([128, KT * 128], bf16, tag="wqk",
                                               name=f"wt_pre_{dt}")
                nc.sync.dma_start(out=wt_pre[0][:], in_=wq[:, 0:H])
                for kt in range(KT):
                    nc.sync.dma_start(out=xt[kt][:], in_=xT[:, kt * S:(kt + 1) * S])
                    if kt == 7:
                        nc.sync.dma_start(out=wt_pre[1][:], in_=wq[:, H:2 * H])
                nc.sync.dma_start(out=wt_pre[2][:], in_=wq[:, 2 * H:3 * H])
                nc.sync.dma_start(out=cos_sb[:], in_=cosT[:])
                if ROPE_BF16:
                    sin_f32 = trig_pool.tile([128, S], f32, tag="sin_f32")
                    nc.sync.dma_start(out=sin_f32[:], in_=sinTs[:])
                    nc.vector.tensor_copy(sin_sb[:], sin_f32[:])
                else:
                    nc.sync.dma_start(out=sin_sb[:], in_=sinTs[:])

                def rope_evict(psum, dst_ap, sc):
                    # dst = psum*cos + shifted(psum)*sin_signed over this chunk
                    cs = cos_sb[:, sc * SCHUNK:(sc + 1) * SCHUNK]
                    ss = sin_sb[:, sc * SCHUNK:(sc + 1) * SCHUNK]
                    m1 = rope_pool.tile([128, SCHUNK], ropedt, tag="rope_m1")
                    nc.vector.tensor_mul(m1[:], psum[:], cs)
                    tmp = rope_pool.tile([128, SCHUNK], ropedt, tag="rope_tmp")
                    for b0 in range(0, 128, 64):
                        src = (b0 + 64) % 128
                        nc.vector.tensor_copy(tmp[b0:b0 + 64, :], psum[src:src + 64, :])
                    nc.vector.tensor_mul(tmp[:], tmp[:], ss)
                    nc.vector.tensor_add(dst_ap, m1[:], tmp[:])

                first = KT_MAJOR_START
                for which, wdram, dst_sb in (("q", wq, qT_sb), ("k", wk, kT_sb)):
                    for dt in range(DT):
                        if which == "q" and dt in wt_pre:
                            wt = wt_pre[dt]
                        else:
                            wt = wqk_pool.tile([128, KT * 128], bf16, tag="wqk",
                                               name=f"wt_{which}_{dt}")
                            for c in range(2):
                                nc.sync.dma_start(
                                    out=wt[:, c * 1024:(c + 1) * 1024],
                                    in_=wdram[:, dt * H + c * 1024: dt * H + (c + 1) * 1024])
                        if first:
                            # kt-major over both s-chunks: each kt step needs
                            # only xT k-tile kt, so matmuls chase the DMAs
                            first = False
                            pss = [psA.tile([128, SCHUNK], f32, tag="psA",
                                            name=f"psA_f{sc}") for sc in range(NSC)]
                            for kt in range(KT):
                                for sc in range(NSC):
                                    nc.tensor.matmul(
                                        pss[sc][:],
                                        wt[:, kt * 128:(kt + 1) * 128],
                                        xt[kt][:, sc * SCHUNK:(sc + 1) * SCHUNK],
                                        start=(kt == 0), stop=(kt == KT - 1),
                                    )
                            for sc in range(NSC):
                                dst = dst_sb[:, dt * S + sc * SCHUNK: dt * S + (sc + 1) * SCHUNK]
                                rope_evict(pss[sc], dst, sc)
                            continue
                        for sc in range(NSC):
                            ps = psA.tile([128, SCHUNK], f32, tag="psA")
                            for kt in range(KT):
                                nc.tensor.matmul(
                                    ps[:],
                                    wt[:, kt * 128:(kt + 1) * 128],
                                    xt[kt][:, sc * SCHUNK:(sc + 1) * SCHUNK],
                                    start=(kt == 0), stop=(kt == KT - 1),
                                )
                            dst = dst_sb[:, dt * S + sc * SCHUNK: dt * S + (sc + 1) * SCHUNK]
                            rope_evict(ps, dst, sc)

                # ---- phase A2: V projection (natural layout) ---------------
                wv_pool_cm = tc.tile_pool(name="wv_stream", bufs=2)
                wv_pool = wv_pool_cm.__enter__()
                psA2 = psA
                for nc4 in range(4):
                    wvt = wv_pool.tile([128, KT * 512], bf16, tag="wv")
                    nc.sync.dma_start(out=wvt[:], in_=wv[:, nc4 * KT * 512:(nc4 + 1) * KT * 512])
                    for st in range(ST):
                        ps = psA2.tile([128, SCHUNK], f32, tag="psA")
                        for kt in range(KT):
                            nc.tensor.matmul(
                                ps[:],
                                xt[kt][:, st * 128:(st + 1) * 128],
                                wvt[:, kt * 512:(kt + 1) * 512],
                                start=(kt == 0), stop=(kt == KT - 1),
                            )
                        nc.scalar.copy(v_sb[:, st * H + nc4 * 512: st * H + (nc4 + 1) * 512], ps[:])
                wv_pool_cm.__exit__(None, None, None)

            # ---- phase B: attention -----------------------------------------
            wo_pre_cm = tc.tile_pool(name="wo_pre", bufs=1)
            wo_pre_pool = wo_pre_cm.__enter__()
            from concourse.bass_isa import ReduceOp
            with tc.tile_pool(name="expS", bufs=2) as expS_pool, \
                 tc.tile_pool(name="attnT", bufs=1) as attnT_pool, \
                 tc.tile_pool(name="tree", bufs=1) as tree_pool, \
                 tc.tile_pool(name="esumP", bufs=2) as esum_pool, \
                 tc.tile_pool(name="den", bufs=2) as den_pool, \
                 tc.tile_pool(name="norm", bufs=2) as norm_pool, \
                 tc.tile_pool(name="psS", bufs=2, space="PSUM") as psS, \
                 tc.tile_pool(name="psAV", bufs=3, space="PSUM") as psAV:

                attnT_sb = attnT_pool.tile([128, NH * S], bf16, tag="attnT")
                av_sb = [attnT_pool.tile([128, S], bf16, tag=f"avsb{i}",
                                         name=f"avsb{i}") for i in range(2)]
                wo0 = wo_pre_pool.tile([128, KT * 128], bf16, tag="wo_pre")
                for c in range(2):
                    nc.sync.dma_start(out=wo0[:, c * 1024:(c + 1) * 1024],
                                      in_=wo[:, c * 1024:(c + 1) * 1024])

                esum_of = {}
                den_of = {}
                ps_av_of = {}

                t4_of = {}

                def emit_tree_l1(h, e_sb, i):
                    """Level-1 pairwise add E(2i)+E(2i+1); spread through the
                    kt8 loop so the tree isn't one DVE burst at head end."""
                    E = lambda j: e_sb[:, j * S:(j + 1) * S]
                    t4 = tree_pool.tile([128, S], bf16, tag=f"tree4_{i}",
                                        name=f"t4_{h}_{i}")
                    nc.vector.tensor_add(t4[:], E(2 * i), E(2 * i + 1))
                    t4_of.setdefault(h, []).append(t4)

                def emit_tree_top(h):
                    t4 = t4_of.pop(h)
                    t2 = [tree_pool.tile([128, S], bf16, tag=f"tree2_{i}",
                                         name=f"t2_{h}_{i}") for i in range(2)]
                    for i in range(2):
                        nc.vector.tensor_add(t2[i][:], t4[2 * i][:], t4[2 * i + 1][:])
                    esum = esum_pool.tile([128, S], bf16, tag="esum",
                                          name=f"esum_{h}")
                    nc.vector.tensor_add(esum[:], t2[0][:], t2[1][:])
                    esum_of[h] = esum
                    # cross-partition sum+broadcast on the idle GpSimd engine
                    # (6.9us; pipelined two heads deep, replaces a PE matmul)
                    denb = den_pool.tile([128, S], f32, tag="denb",
                                         name=f"denb_{h}")
                    nc.gpsimd.partition_all_reduce(denb[:], esum[:], 128,
                                                   ReduceOp.add)
                    den_of[h] = denb

                def emit_den_norm(h, qc):
                    """DVE reciprocal of the GpSimd-reduced denominator +
                    normalize from the SBUF-staged AV."""
                    denb = den_of[h]
                    recipb = norm_pool.tile([128, SCHUNK], f32, tag="recipb",
                                            name=f"recipb_{h}_{qc}")
                    nc.vector.reciprocal_approx_fast(
                        out=recipb[:], in_=denb[:, qc * SCHUNK:(qc + 1) * SCHUNK])
                    nc.vector.tensor_mul(
                        attnT_sb[:, h * S + qc * SCHUNK: h * S + (qc + 1) * SCHUNK],
                        av_sb[h % 2][:, qc * SCHUNK:(qc + 1) * SCHUNK],
                        recipb[:])
                    if qc == NSC - 1:
                        del esum_of[h]
                        del den_of[h]
                        del ps_av_of[h]

                expS_of = {}

                def emit_scores_exp(h, kt8):
                    e_sb = expS_of[h]
                    ps_s = psS.tile([128, S], f32, tag="psS",
                                    name=f"pss_{h}_{kt8}")
                    for qc in range(NSC):
                        nc.tensor.matmul(
                            ps_s[:, qc * SCHUNK:(qc + 1) * SCHUNK],
                            kT_sb[:, h * S + kt8 * 128: h * S + (kt8 + 1) * 128],
                            qT_sb[:, h * S + qc * SCHUNK: h * S + (qc + 1) * SCHUNK],
                            start=True, stop=True,
                        )
                    nc.scalar.activation(
                        e_sb[:, kt8 * S:(kt8 + 1) * S], ps_s[:],
                        func=mybir.ActivationFunctionType.Exp,
                        scale=SCALE,
                    )

                expS_of[0] = expS_pool.tile([128, ST * S], bf16, tag="expS",
                                            name="expS_0")
                for h in range(NH):
                    expS_sb = expS_of[h]
                    ps_av_of[h] = [psAV.tile([128, SCHUNK], f32, tag="psAV",
                                             name=f"psav_{h}_{qc}") for qc in range(NSC)]
                    ps_av = ps_av_of[h]
                    for kt8 in range(ST):
                        # kt8=0 of heads > 0 was pulled into the previous
                        # head's tail so ScalarE never idles at the boundary
                        if kt8 > 0 or h == 0:
                            emit_scores_exp(h, kt8)
                        # spread tree level-1 adds through the loop (first 3)
                        if kt8 in (1, 3, 5):
                            emit_tree_l1(h, expS_sb, kt8 // 2)
                        # denominator chain pipelined two heads back (the
                        # GpSimd reduce of head h-1 is still in flight)
                        if h > 1 and kt8 in (4, 5):
                            emit_den_norm(h - 2, kt8 - 4)
                        if kt8 == ST - 1 and h + 1 < NH:
                            expS_of[h + 1] = expS_pool.tile(
                                [128, ST * S], bf16, tag="expS",
                                name=f"expS_{h + 1}")
                            emit_scores_exp(h + 1, 0)
                        for qc in range(NSC):
                            nc.tensor.matmul(
                                ps_av[qc][:],
                                v_sb[:, kt8 * H + h * 128: kt8 * H + (h + 1) * 128],
                                expS_sb[:, kt8 * S + qc * SCHUNK: kt8 * S + (qc + 1) * SCHUNK],
                                start=(kt8 == 0), stop=(kt8 == ST - 1),
                            )
                    # stage AV psum to SBUF on DVE (frees the psum pair for
                    # the next head), then finish the tree + GpSimd reduce
                    for qc in range(NSC):
                        nc.vector.tensor_copy(
                            av_sb[h % 2][:, qc * SCHUNK:(qc + 1) * SCHUNK],
                            ps_av[qc][:])
                    emit_tree_l1(h, expS_sb, 3)
                    emit_tree_top(h)

                for hh in (NH - 2, NH - 1):
                    for qc in range(NSC):
                        emit_den_norm(hh, qc)

            # ---- phase C: O-projection -------------------------------------
            with tc.tile_pool(name="wo_stream", bufs=4) as wo_pool, \
                 tc.tile_pool(name="ostage", bufs=2) as ostage_pool, \
                 tc.tile_pool(name="psO", bufs=2, space="PSUM") as psO:
                for ot in range(DT):
                    if ot == 0:
                        wot = wo0
                    else:
                        wot = wo_pool.tile([128, KT * 128], bf16, tag="wo",
                                           name=f"wo_{ot}")
                        nc.sync.dma_start(out=wot[:], in_=wo[:, ot * H:(ot + 1) * H])
                    ps_o = psO.tile([128, S], f32, tag="psO")
                    for dt in range(DT):
                        for qc in range(NSC):
                            nc.tensor.matmul(
                                ps_o[:, qc * SCHUNK:(qc + 1) * SCHUNK],
                                wot[:, dt * 128:(dt + 1) * 128],
                                attnT_sb[:, dt * S + qc * SCHUNK: dt * S + (qc + 1) * SCHUNK],
                                start=(dt == 0), stop=(dt == DT - 1),
                            )
                    o_sb = ostage_pool.tile([128, S], f32, tag="ostage")
                    # per-chunk evict+DMA so the final chunk's copy overlaps
                    # the previous chunk's store (shorter kernel tail)
                    for c in range(2):
                        nc.scalar.copy(o_sb[:, c * SCHUNK:(c + 1) * SCHUNK],
                                       ps_o[:, c * SCHUNK:(c + 1) * SCHUNK])
                        nc.sync.dma_start(
                            out=out[ot * 128:(ot + 1) * 128, c * SCHUNK:(c + 1) * SCHUNK],
                            in_=o_sb[:, c * SCHUNK:(c + 1) * SCHUNK])
            wo_pre_cm.__exit__(None, None, None)
    nc.finalize()
    return nc


def _prep_core_inputs(x_bc, wq_t, wk_t, wv_t, wo_t, cosT_p, sinTs_p):
    # x_bc: (S, H) f32 -> xT partition-major [128, KT*S] bf16
    xT_p = np.ascontiguousarray(
        x_bc.T.reshape(KT, 128, S).transpose(1, 0, 2).reshape(128, KT * S)
    ).astype(BF)
    return {
        "xT": xT_p, "wq": wq_t, "wk": wk_t, "wv": wv_t, "wo": wo_t,
        "cosT": cosT_p, "sinTs": sinTs_p,
    }


def _prep_shared(cos, sin, w_qkv, w_o):
    def dtile_major(w):  # (H, 2048) -> [128, DT*H], lhsT tiles (dt, kt)
        return np.ascontiguousarray(
            w.reshape(KT, 128, DT, 128).transpose(1, 2, 0, 3).reshape(128, DT * H)
        ).astype(BF)

    wq_t = dtile_major(w_qkv[:, :H])
    wk_t = dtile_major(w_qkv[:, H:2 * H])
    wo_t = dtile_major(w_o)
    wv_t = np.ascontiguousarray(
        w_qkv[:, 2 * H:].reshape(KT, 128, 4, 512).transpose(1, 2, 0, 3)
        .reshape(128, 4 * KT * 512)
    ).astype(BF)

    cos_p = np.ones((S, HD), np.float32)
    cos_p[:NENC] = cos
    sin_p = np.zeros((S, HD), np.float32)
    sin_p[:NENC] = sin
    cosT_p = np.ascontiguousarray(cos_p.T)
    sinT = sin_p.T.copy()
    sinTs_p = np.concatenate([-sinT[:64], sinT[64:]], axis=0)
    sinTs_p = np.ascontiguousarray(sinTs_p)
    return wq_t, wk_t, wv_t, wo_t, cosT_p, sinTs_p


_CACHED_NC = None


def kernel(hidden_states, cos, sin, w_qkv, w_o):
    global _CACHED_NC
    from concourse.bass_utils import run_bass_kernel_spmd

    hidden_states = np.asarray(hidden_states, dtype=np.float32)
    cos = np.asarray(cos, dtype=np.float32)
    sin = np.asarray(sin, dtype=np.float32)
    w_qkv = np.asarray(w_qkv, dtype=np.float32)
    w_o = np.asarray(w_o, dtype=np.float32)

    shared = _prep_shared(cos, sin, w_qkv, w_o)
    xs = hidden_states.reshape(B * C, S, H)
    in_maps = [_prep_core_inputs(xs[i], *shared) for i in range(NCORES)]

    if _CACHED_NC is None:
        _CACHED_NC = build_nc()
    res = run_bass_kernel_spmd(_CACHED_NC, in_maps, list(range(NCORES)))

    out_full = np.empty((B * C, S, H), np.float32)
    for i in range(NCORES):
        out_full[i] = np.asarray(res.results[i]["out"], dtype=np.float32).T
    return out_full.reshape(B, C, S, H)


# revision 35
# speedup vs baseline: 1.0002x; 1.0002x over previous
"""Trainium2 Bass kernel: multi-head attention (B,C,S,H)=(2,4,1024,2048), NH=16, HD=128.

Strategy: pure data-parallel over the 8 B*C batch elements -> 8 NeuronCores,
no collectives.  Per core, 3 phases (power-aware: the chip P0-downclocks to
~2.0 GHz under sustained high engine activity, so the schedule keeps total
non-PE engine work low rather than maximally dense):
  phase A: Q^T,K^T projection (transposed [head-dim, seq] layout) with RoPE
           fused into the PSUM eviction (bf16 arithmetic where possible).
           The first d-tile runs kt-major so matmuls chase the xT DMA stream.
  phase A2: V projection (natural [seq, head-dim] layout).
  phase B: per-head attention: scores^T -> ScalarE exp -> AV matmul; softmax
           denominator (DVE tree-sum + ones-matmul broadcast + reciprocal)
           optionally pipelined one head back so the PE never waits on it.
  phase C: O-projection from resident attn^T tiles, streamed w_o.
All matmuls bf16; f32 accumulation in PSUM.  Host pre-transposes/pre-tiles/
casts inputs so every DMA is partition-major contiguous.
"""

import numpy as np
import ml_dtypes

try:
    import concourse  # noqa: F401
except ImportError:
    import sys
    sys.path.insert(0, "/opt/trn_rl_repo")

BF = ml_dtypes.bfloat16

B, C, S, H = 2, 4, 1024, 2048
NH, HD, NENC = 16, 128, 1008
NCORES = 8
KT = H // 128          # 16 contraction tiles for the projections
DT = H // 128          # 16 output d-tiles (heads) for Q/K
ST = S // 128          # 8 seq tiles
SCHUNK = 512
NSC = S // SCHUNK      # 2 seq chunks
SCALE = 1.0 / float(np.sqrt(HD))

ROPE_BF16 = True       # bf16 rope mul/add (less DVE activity)
KT_MAJOR_START = True  # first Q d-tile kt-major to chase the xT DMA stream
PIPELINE_DEN = True    # issue denominator of head h-1 inside head h's loop


def build_nc():
    import concourse.bass as bass
    import concourse.mybir as mybir
    import concourse.tile as tile
    from concourse import bacc

    f32 = mybir.dt.float32
    bf16 = mybir.dt.bfloat16
    ropedt = bf16 if ROPE_BF16 else f32

    nc = bacc.Bacc(None, target_bir_lowering=False, debug=False)

    xT = nc.dram_tensor("xT", [128, KT * S], bf16, kind="ExternalInput")
    wq = nc.dram_tensor("wq", [128, DT * H], bf16, kind="ExternalInput")
    wk = nc.dram_tensor("wk", [128, DT * H], bf16, kind="ExternalInput")
    wv = nc.dram_tensor("wv", [128, 4 * KT * 512], bf16, kind="ExternalInput")
    wo = nc.dram_tensor("wo", [128, DT * H], bf16, kind="ExternalInput")
    cosT = nc.dram_tensor("cosT", [128, S], f32, kind="ExternalInput")
    sinTs = nc.dram_tensor("sinTs", [128, S], f32, kind="ExternalInput")
    out = nc.dram_tensor("out", [H, S], f32, kind="ExternalOutput")

    with tile.TileContext(nc) as tc:
        import contextlib
        with contextlib.ExitStack() as ctx:
            # ---- persistent SBUF tiles -------------------------------------
            persist = ctx.enter_context(tc.tile_pool(name="persist", bufs=1))
            qT_sb = persist.tile([128, NH * S], bf16, tag="qT")
            kT_sb = persist.tile([128, NH * S], bf16, tag="kT")
            v_sb = persist.tile([128, ST * H], bf16, tag="v")

            # ---- phase A: Q^T / K^T projection with fused RoPE -------------
            with tc.tile_pool(name="xpool", bufs=1) as x_pool, \
                 tc.tile_pool(name="trig", bufs=1) as trig_pool, \
                 tc.tile_pool(name="wqk_stream", bufs=3) as wqk_pool, \
                 tc.tile_pool(name="rope_scratch", bufs=2) as rope_pool, \
                 tc.tile_pool(name="psumA", bufs=4, space="PSUM") as psA:

                cos_sb = trig_pool.tile([128, S], f32, tag="cos")
                sin_sb = trig_pool.tile([128, S], ropedt, tag="sin")
                # xT as 16 per-kt tiles: Tile tracks dependencies per tile, so
                # matmuls gate on their own k-tile's DMA, not the full 4.2MB
                xt = [x_pool.tile([128, S], bf16, tag=f"xT{kt}", name=f"xT{kt}")
                      for kt in range(KT)]
                wt_pre = {}
                for dt in range(3):
                    wt_pre[dt] = wqk_pool.tile([128, KT * 128], bf16, tag="wqk",
                                               name=f"wt_pre_{dt}")
                nc.sync.dma_start(out=wt_pre[0][:], in_=wq[:, 0:H])
                for kt in range(KT):
                    nc.sync.dma_start(out=xt[kt][:], in_=xT[:, kt * S:(kt + 1) * S])
                    if kt == 7:
                        nc.sync.dma_start(out=wt_pre[1][:], in_=wq[:, H:2 * H])
                nc.sync.dma_start(out=wt_pre[2][:], in_=wq[:, 2 * H:3 * H])
                nc.sync.dma_start(out=cos_sb[:], in_=cosT[:])
                if ROPE_BF16:
                    sin_f32 = trig_pool.tile([128, S], f32, tag="sin_f32")
                    nc.sync.dma_start(out=sin_f32[:], in_=sinTs[:])
                    nc.vector.tensor_copy(sin_sb[:], sin_f32[:])
                else:
                    nc.sync.dma_start(out=sin_sb[:], in_=sinTs[:])

                def rope_evict(psum, dst_ap, sc):
                    # dst = psum*cos + shifted(psum)*sin_signed over this chunk
                    cs = cos_sb[:, sc * SCHUNK:(sc + 1) * SCHUNK]
                    ss = sin_sb[:, sc * SCHUNK:(sc + 1) * SCHUNK]
                    m1 = rope_pool.tile([128, SCHUNK], ropedt, tag="rope_m1")
                    nc.vector.tensor_mul(m1[:], psum[:], cs)
                    tmp = rope_pool.tile([128, SCHUNK], ropedt, tag="rope_tmp")
                    for b0 in range(0, 128, 64):
                        src = (b0 + 64) % 128
                        nc.vector.tensor_copy(tmp[b0:b0 + 64, :], psum[src:src + 64, :])
                    nc.vector.tensor_mul(tmp[:], tmp[:], ss)
                    nc.vector.tensor_add(dst_ap, m1[:], tmp[:])

                first = KT_MAJOR_START
                for which, wdram, dst_sb in (("q", wq, qT_sb), ("k", wk, kT_sb)):
                    for dt in range(DT):
                        if which == "q" and dt in wt_pre:
                            wt = wt_pre[dt]
                        else:
                            wt = wqk_pool.tile([128, KT * 128], bf16, tag="wqk",
                                               name=f"wt_{which}_{dt}")
                            for c in range(2):
                                nc.sync.dma_start(
                                    out=wt[:, c * 1024:(c + 1) * 1024],
                                    in_=wdram[:, dt * H + c * 1024: dt * H + (c + 1) * 1024])
                        if first:
                            # kt-major over both s-chunks: each kt step needs
                            # only xT k-tile kt, so matmuls chase the DMAs
                            first = False
                            pss = [psA.tile([128, SCHUNK], f32, tag="psA",
                                            name=f"psA_f{sc}") for sc in range(NSC)]
                            for kt in range(KT):
                                for sc in range(NSC):
                                    nc.tensor.matmul(
                                        pss[sc][:],
                                        wt[:, kt * 128:(kt + 1) * 128],
                                        xt[kt][:, sc * SCHUNK:(sc + 1) * SCHUNK],
                                        start=(kt == 0), stop=(kt == KT - 1),
                                    )
                            for sc in range(NSC):
                                dst = dst_sb[:, dt * S + sc * SCHUNK: dt * S + (sc + 1) * SCHUNK]
                                rope_evict(pss[sc], dst, sc)
                            continue
                        for sc in range(NSC):
                            ps = psA.tile([128, SCHUNK], f32, tag="psA")
                            for kt in range(KT):
                                nc.tensor.matmul(
                                    ps[:],
                                    wt[:, kt * 128:(kt + 1) * 128],
                                    xt[kt][:, sc * SCHUNK:(sc + 1) * SCHUNK],
                                    start=(kt == 0), stop=(kt == KT - 1),
                                )
                            dst = dst_sb[:, dt * S + sc * SCHUNK: dt * S + (sc + 1) * SCHUNK]
                            rope_evict(ps, dst, sc)

                # ---- phase A2: V projection (natural layout) ---------------
                wv_pool_cm = tc.tile_pool(name="wv_stream", bufs=2)
                wv_pool = wv_pool_cm.__enter__()
                psA2 = psA
                for nc4 in range(4):
                    wvt = wv_pool.tile([128, KT * 512], bf16, tag="wv")
                    nc.sync.dma_start(out=wvt[:], in_=wv[:, nc4 * KT * 512:(nc4 + 1) * KT * 512])
                    for st in range(ST):
                        ps = psA2.tile([128, SCHUNK], f32, tag="psA")
                        for kt in range(KT):
                            nc.tensor.matmul(
                                ps[:],
                                xt[kt][:, st * 128:(st + 1) * 128],
                                wvt[:, kt * 512:(kt + 1) * 512],
                                start=(kt == 0), stop=(kt == KT - 1),
                            )
                        nc.scalar.copy(v_sb[:, st * H + nc4 * 512: st * H + (nc4 + 1) * 512], ps[:])
                wv_pool_cm.__exit__(None, None, None)

            # ---- phase B: attention -----------------------------------------
            wo_pre_cm = tc.tile_pool(name="wo_pre", bufs=1)
            wo_pre_pool = wo_pre_cm.__enter__()
            from concourse.bass_isa import ReduceOp
            with tc.tile_pool(name="expS", bufs=2) as expS_pool, \
                 tc.tile_pool(name="attnT", bufs=1) as attnT_pool, \
                 tc.tile_pool(name="tree", bufs=1) as tree_pool, \
                 tc.tile_pool(name="esumP", bufs=2) as esum_pool, \
                 tc.tile_pool(name="den", bufs=2) as den_pool, \
                 tc.tile_pool(name="norm", bufs=2) as norm_pool, \
                 tc.tile_pool(name="psS", bufs=2, space="PSUM") as psS, \
                 tc.tile_pool(name="psAV", bufs=3, space="PSUM") as psAV:

                attnT_sb = attnT_pool.tile([128, NH * S], bf16, tag="attnT")
                av_sb = [attnT_pool.tile([128, S], bf16, tag=f"avsb{i}",
                                         name=f"avsb{i}") for i in range(3)]
                wo0 = wo_pre_pool.tile([128, KT * 128], bf16, tag="wo_pre")
                for c in range(2):
                    nc.sync.dma_start(out=wo0[:, c * 1024:(c + 1) * 1024],
                                      in_=wo[:, c * 1024:(c + 1) * 1024])

                esum_of = {}
                den_of = {}
                ps_av_of = {}

                t4_of = {}

                def emit_tree_l1(h, e_sb, i):
                    """Level-1 pairwise add E(2i)+E(2i+1); spread through the
                    kt8 loop so the tree isn't one DVE burst at head end."""
                    E = lambda j: e_sb[:, j * S:(j + 1) * S]
                    t4 = tree_pool.tile([128, S], bf16, tag=f"tree4_{i}",
                                        name=f"t4_{h}_{i}")
                    nc.vector.tensor_add(t4[:], E(2 * i), E(2 * i + 1))
                    t4_of.setdefault(h, []).append(t4)

                def emit_tree_top(h):
                    t4 = t4_of.pop(h)
                    t2 = [tree_pool.tile([128, S], bf16, tag=f"tree2_{i}",
                                         name=f"t2_{h}_{i}") for i in range(2)]
                    for i in range(2):
                        nc.vector.tensor_add(t2[i][:], t4[2 * i][:], t4[2 * i + 1][:])
                    esum = esum_pool.tile([128, S], bf16, tag="esum",
                                          name=f"esum_{h}")
                    nc.vector.tensor_add(esum[:], t2[0][:], t2[1][:])
                    esum_of[h] = esum
                    # cross-partition sum+broadcast on the idle GpSimd engine
                    # (6.9us; pipelined two heads deep, replaces a PE matmul)
                    denb = den_pool.tile([128, S], f32, tag="denb",
                                         name=f"denb_{h}")
                    nc.gpsimd.partition_all_reduce(denb[:], esum[:], 128,
                                                   ReduceOp.add)
                    den_of[h] = denb

                def emit_den_norm(h, qc):
                    """DVE reciprocal of the GpSimd-reduced denominator +
                    normalize from the SBUF-staged AV."""
                    denb = den_of[h]
                    recipb = norm_pool.tile([128, SCHUNK], f32, tag="recipb",
                                            name=f"recipb_{h}_{qc}")
                    nc.vector.reciprocal_approx_fast(
                        out=recipb[:], in_=denb[:, qc * SCHUNK:(qc + 1) * SCHUNK])
                    nc.vector.tensor_mul(
                        attnT_sb[:, h * S + qc * SCHUNK: h * S + (qc + 1) * SCHUNK],
                        av_sb[h % 3][:, qc * SCHUNK:(qc + 1) * SCHUNK],
                        recipb[:])
                    if qc == NSC - 1:
                        del esum_of[h]
                        del den_of[h]
                        del ps_av_of[h]

                expS_of = {}

                def emit_scores_exp(h, kt8):
                    e_sb = expS_of[h]
                    ps_s = psS.tile([128, S], f32, tag="psS",
                                    name=f"pss_{h}_{kt8}")
                    for qc in range(NSC):
                        nc.tensor.matmul(
                            ps_s[:, qc * SCHUNK:(qc + 1) * SCHUNK],
                            kT_sb[:, h * S + kt8 * 128: h * S + (kt8 + 1) * 128],
                            qT_sb[:, h * S + qc * SCHUNK: h * S + (qc + 1) * SCHUNK],
                            start=True, stop=True,
                        )
                    nc.scalar.activation(
                        e_sb[:, kt8 * S:(kt8 + 1) * S], ps_s[:],
                        func=mybir.ActivationFunctionType.Exp,
                        scale=SCALE,
                    )

                expS_of[0] = expS_pool.tile([128, ST * S], bf16, tag="expS",
                                            name="expS_0")
                for h in range(NH):
                    expS_sb = expS_of[h]
                    ps_av_of[h] = [psAV.tile([128, SCHUNK], f32, tag="psAV",
                                             name=f"psav_{h}_{qc}") for qc in range(NSC)]
                    ps_av = ps_av_of[h]
                    for kt8 in range(ST):
                        # kt8=0 of heads > 0 was pulled into the previous
                        # head's tail so ScalarE never idles at the boundary
                        if kt8 > 0 or h == 0:
                            emit_scores_exp(h, kt8)
                        # spread tree level-1 adds through the loop (first 3)
                        if kt8 in (1, 3, 5):
                            emit_tree_l1(h, expS_sb, kt8 // 2)
                        if kt8 == ST - 1 and h + 1 < NH:
                            expS_of[h + 1] = expS_pool.tile(
                                [128, ST * S], bf16, tag="expS",
                                name=f"expS_{h + 1}")
                            emit_scores_exp(h + 1, 0)
                        for qc in range(NSC):
                            nc.tensor.matmul(
                                ps_av[qc][:],
                                v_sb[:, kt8 * H + h * 128: kt8 * H + (h + 1) * 128],
                                expS_sb[:, kt8 * S + qc * SCHUNK: kt8 * S + (qc + 1) * SCHUNK],
                                start=(kt8 == 0), stop=(kt8 == ST - 1),
                            )
                    # stage AV psum to SBUF on DVE (frees the psum pair for
                    # the next head), then finish the tree + GpSimd reduce
                    for qc in range(NSC):
                        nc.vector.tensor_copy(
                            av_sb[h % 3][:, qc * SCHUNK:(qc + 1) * SCHUNK],
                            ps_av[qc][:])
                    emit_tree_l1(h, expS_sb, 3)
                    emit_tree_top(h)
                    # denominator chain pipelined two heads back, emitted
                    # last so the DVE FIFO never blocks on the GpSimd reduce
                    if h > 1:
                        for qc in range(NSC):
                            emit_den_norm(h - 2, qc)

                for hh in (NH - 2, NH - 1):
                    for qc in range(NSC):
                        emit_den_norm(hh, qc)

            # ---- phase C: O-projection -------------------------------------
            with tc.tile_pool(name="wo_stream", bufs=4) as wo_pool, \
                 tc.tile_pool(name="ostage", bufs=2) as ostage_pool, \
                 tc.tile_pool(name="psO", bufs=2, space="PSUM") as psO:
                for ot in range(DT):
                    if ot == 0:
                        wot = wo0
                    else:
                        wot = wo_pool.tile([128, KT * 128], bf16, tag="wo",
                                           name=f"wo_{ot}")
                        nc.sync.dma_start(out=wot[:], in_=wo[:, ot * H:(ot + 1) * H])
                    ps_o = psO.tile([128, S], f32, tag="psO")
                    for dt in range(DT):
                        for qc in range(NSC):
                            nc.tensor.matmul(
                                ps_o[:, qc * SCHUNK:(qc + 1) * SCHUNK],
                                wot[:, dt * 128:(dt + 1) * 128],
                                attnT_sb[:, dt * S + qc * SCHUNK: dt * S + (qc + 1) * SCHUNK],
                                start=(dt == 0), stop=(dt == DT - 1),
                            )
                    o_sb = ostage_pool.tile([128, S], f32, tag="ostage")
                    # per-chunk evict+DMA so the final chunk's copy overlaps
                    # the previous chunk's store (shorter kernel tail)
                    for c in range(2):
                        nc.scalar.copy(o_sb[:, c * SCHUNK:(c + 1) * SCHUNK],
                                       ps_o[:, c * SCHUNK:(c + 1) * SCHUNK])
                        nc.sync.dma_start(
                            out=out[ot * 128:(ot + 1) * 128, c * SCHUNK:(c + 1) * SCHUNK],
                            in_=o_sb[:, c * SCHUNK:(c + 1) * SCHUNK])
            wo_pre_cm.__exit__(None, None, None)
    nc.finalize()
    return nc


def _prep_core_inputs(x_bc, wq_t, wk_t, wv_t, wo_t, cosT_p, sinTs_p):
    # x_bc: (S, H) f32 -> xT partition-major [128, KT*S] bf16
    xT_p = np.ascontiguousarray(
        x_bc.T.reshape(KT, 128, S).transpose(1, 0, 2).reshape(128, KT * S)
    ).astype(BF)
    return {
        "xT": xT_p, "wq": wq_t, "wk": wk_t, "wv": wv_t, "wo": wo_t,
        "cosT": cosT_p, "sinTs": sinTs_p,
    }


def _prep_shared(cos, sin, w_qkv, w_o):
    def dtile_major(w):  # (H, 2048) -> [128, DT*H], lhsT tiles (dt, kt)
        return np.ascontiguousarray(
            w.reshape(KT, 128, DT, 128).transpose(1, 2, 0, 3).reshape(128, DT * H)
        ).astype(BF)

    wq_t = dtile_major(w_qkv[:, :H])
    wk_t = dtile_major(w_qkv[:, H:2 * H])
    wo_t = dtile_major(w_o)
    wv_t = np.ascontiguousarray(
        w_qkv[:, 2 * H:].reshape(KT, 128, 4, 512).transpose(1, 2, 0, 3)
        .reshape(128, 4 * KT * 512)
    ).astype(BF)

    cos_p = np.ones((S, HD), np.float32)
    cos_p[:NENC] = cos
    sin_p = np.zeros((S, HD), np.float32)
    sin_p[:NENC] = sin
    cosT_p = np.ascontiguousarray(cos_p.T)
    sinT = sin_p.T.copy()
    sinTs_p = np.concatenate([-sinT[:64], sinT[64:]], axis=0)
    sinTs_p = np.ascontiguousarray(sinTs_p)
    return wq_t, wk_t, wv_t, wo_t, cosT_p, sinTs_p


_CACHED_NC = None


def kernel(hidden_states, cos, sin, w_qkv, w_o):
    global _CACHED_NC
    from concourse.bass_utils import run_bass_kernel_spmd

    hidden_states = np.asarray(hidden_states, dtype=np.float32)
    cos = np.asarray(cos, dtype=np.float32)
    sin = np.asarray(sin, dtype=np.float32)
    w_qkv = np.asarray(w_qkv, dtype=np.float32)
    w_o = np.asarray(w_o, dtype=np.float32)

    shared = _prep_shared(cos, sin, w_qkv, w_o)
    xs = hidden_states.reshape(B * C, S, H)
    in_maps = [_prep_core_inputs(xs[i], *shared) for i in range(NCORES)]

    if _CACHED_NC is None:
        _CACHED_NC = build_nc()
    res = run_bass_kernel_spmd(_CACHED_NC, in_maps, list(range(NCORES)))

    out_full = np.empty((B * C, S, H), np.float32)
    for i in range(NCORES):
        out_full[i] = np.asarray(res.results[i]["out"], dtype=np.float32).T
    return out_full.reshape(B, C, S, H)


# revision 36
# speedup vs baseline: 1.1488x; 1.1485x over previous
"""Trainium2 Bass kernel: multi-head attention (B,C,S,H)=(2,4,1024,2048), NH=16, HD=128.

Strategy: pure data-parallel over the 8 B*C batch elements -> 8 NeuronCores,
no collectives.  Per core, 3 phases (power-aware: the chip P0-downclocks to
~2.0 GHz under sustained high engine activity, so the schedule keeps total
non-PE engine work low rather than maximally dense):
  phase A: Q^T,K^T projection (transposed [head-dim, seq] layout) with RoPE
           fused into the PSUM eviction (bf16 arithmetic where possible).
           The first d-tile runs kt-major so matmuls chase the xT DMA stream.
  phase A2: V projection (natural [seq, head-dim] layout).
  phase B: per-head attention: scores^T -> ScalarE exp -> AV matmul; softmax
           denominator (DVE tree-sum + ones-matmul broadcast + reciprocal)
           optionally pipelined one head back so the PE never waits on it.
  phase C: O-projection from resident attn^T tiles, streamed w_o.
All matmuls bf16; f32 accumulation in PSUM.  Host pre-transposes/pre-tiles/
casts inputs so every DMA is partition-major contiguous.
"""

import numpy as np
import ml_dtypes

try:
    import concourse  # noqa: F401
except ImportError:
    import sys
    sys.path.insert(0, "/opt/trn_rl_repo")

BF = ml_dtypes.bfloat16

B, C, S, H = 2, 4, 1024, 2048
NH, HD, NENC = 16, 128, 1008
NCORES = 8
KT = H // 128          # 16 contraction tiles for the projections
DT = H // 128          # 16 output d-tiles (heads) for Q/K
ST = S // 128          # 8 seq tiles
SCHUNK = 512
NSC = S // SCHUNK      # 2 seq chunks
SCALE = 1.0 / float(np.sqrt(HD))

ROPE_BF16 = True       # bf16 rope mul/add (less DVE activity)
KT_MAJOR_START = True  # first Q d-tile kt-major to chase the xT DMA stream
PIPELINE_DEN = True    # issue denominator of head h-1 inside head h's loop


def build_nc():
    import concourse.bass as bass
    import concourse.mybir as mybir
    import concourse.tile as tile
    from concourse import bacc

    f32 = mybir.dt.float32
    bf16 = mybir.dt.bfloat16
    ropedt = bf16 if ROPE_BF16 else f32

    nc = bacc.Bacc(None, target_bir_lowering=False, debug=False)

    xT = nc.dram_tensor("xT", [128, KT * S], bf16, kind="ExternalInput")
    wq = nc.dram_tensor("wq", [128, DT * H], bf16, kind="ExternalInput")
    wk = nc.dram_tensor("wk", [128, DT * H], bf16, kind="ExternalInput")
    wv = nc.dram_tensor("wv", [128, 4 * KT * 512], bf16, kind="ExternalInput")
    wo = nc.dram_tensor("wo", [128, DT * H], bf16, kind="ExternalInput")
    cosT = nc.dram_tensor("cosT", [128, S], f32, kind="ExternalInput")
    sinTs = nc.dram_tensor("sinTs", [128, S], f32, kind="ExternalInput")
    out = nc.dram_tensor("out", [H, S], f32, kind="ExternalOutput")

    with tile.TileContext(nc) as tc:
        import contextlib
        with contextlib.ExitStack() as ctx:
            # ---- persistent SBUF tiles -------------------------------------
            persist = ctx.enter_context(tc.tile_pool(name="persist", bufs=1))
            qT_sb = persist.tile([128, NH * S], bf16, tag="qT")
            kT_sb = persist.tile([128, NH * S], bf16, tag="kT")
            v_sb = persist.tile([128, ST * H], bf16, tag="v")
            ones_mat = persist.tile([128, 128], bf16, tag="ones_mat")
            nc.vector.memset(ones_mat[:], 1.0)

            # ---- phase A: Q^T / K^T projection with fused RoPE -------------
            with tc.tile_pool(name="xpool", bufs=1) as x_pool, \
                 tc.tile_pool(name="trig", bufs=1) as trig_pool, \
                 tc.tile_pool(name="wqk_stream", bufs=3) as wqk_pool, \
                 tc.tile_pool(name="rope_scratch", bufs=2) as rope_pool, \
                 tc.tile_pool(name="psumA", bufs=4, space="PSUM") as psA:

                cos_sb = trig_pool.tile([128, S], f32, tag="cos")
                sin_sb = trig_pool.tile([128, S], ropedt, tag="sin")
                # xT as 16 per-kt tiles: Tile tracks dependencies per tile, so
                # matmuls gate on their own k-tile's DMA, not the full 4.2MB
                xt = [x_pool.tile([128, S], bf16, tag=f"xT{kt}", name=f"xT{kt}")
                      for kt in range(KT)]
                wt_pre = {}
                for dt in range(3):
                    wt_pre[dt] = wqk_pool.tile([128, KT * 128], bf16, tag="wqk",
                                               name=f"wt_pre_{dt}")
                nc.sync.dma_start(out=wt_pre[0][:], in_=wq[:, 0:H])
                for kt in range(KT):
                    nc.sync.dma_start(out=xt[kt][:], in_=xT[:, kt * S:(kt + 1) * S])
                    if kt == 7:
                        nc.sync.dma_start(out=wt_pre[1][:], in_=wq[:, H:2 * H])
                nc.sync.dma_start(out=wt_pre[2][:], in_=wq[:, 2 * H:3 * H])
                nc.sync.dma_start(out=cos_sb[:], in_=cosT[:])
                if ROPE_BF16:
                    sin_f32 = trig_pool.tile([128, S], f32, tag="sin_f32")
                    nc.sync.dma_start(out=sin_f32[:], in_=sinTs[:])
                    nc.vector.tensor_copy(sin_sb[:], sin_f32[:])
                else:
                    nc.sync.dma_start(out=sin_sb[:], in_=sinTs[:])

                def rope_evict(psum, dst_ap, sc):
                    # dst = psum*cos + shifted(psum)*sin_signed over this chunk
                    cs = cos_sb[:, sc * SCHUNK:(sc + 1) * SCHUNK]
                    ss = sin_sb[:, sc * SCHUNK:(sc + 1) * SCHUNK]
                    m1 = rope_pool.tile([128, SCHUNK], ropedt, tag="rope_m1")
                    nc.vector.tensor_mul(m1[:], psum[:], cs)
                    tmp = rope_pool.tile([128, SCHUNK], ropedt, tag="rope_tmp")
                    for b0 in range(0, 128, 64):
                        src = (b0 + 64) % 128
                        nc.vector.tensor_copy(tmp[b0:b0 + 64, :], psum[src:src + 64, :])
                    nc.vector.tensor_mul(tmp[:], tmp[:], ss)
                    nc.vector.tensor_add(dst_ap, m1[:], tmp[:])

                first = KT_MAJOR_START
                for which, wdram, dst_sb in (("q", wq, qT_sb), ("k", wk, kT_sb)):
                    for dt in range(DT):
                        if which == "q" and dt in wt_pre:
                            wt = wt_pre[dt]
                        else:
                            wt = wqk_pool.tile([128, KT * 128], bf16, tag="wqk",
                                               name=f"wt_{which}_{dt}")
                            for c in range(2):
                                nc.sync.dma_start(
                                    out=wt[:, c * 1024:(c + 1) * 1024],
                                    in_=wdram[:, dt * H + c * 1024: dt * H + (c + 1) * 1024])
                        if first:
                            # kt-major over both s-chunks: each kt step needs
                            # only xT k-tile kt, so matmuls chase the DMAs
                            first = False
                            pss = [psA.tile([128, SCHUNK], f32, tag="psA",
                                            name=f"psA_f{sc}") for sc in range(NSC)]
                            for kt in range(KT):
                                for sc in range(NSC):
                                    nc.tensor.matmul(
                                        pss[sc][:],
                                        wt[:, kt * 128:(kt + 1) * 128],
                                        xt[kt][:, sc * SCHUNK:(sc + 1) * SCHUNK],
                                        start=(kt == 0), stop=(kt == KT - 1),
                                    )
                            for sc in range(NSC):
                                dst = dst_sb[:, dt * S + sc * SCHUNK: dt * S + (sc + 1) * SCHUNK]
                                rope_evict(pss[sc], dst, sc)
                            continue
                        for sc in range(NSC):
                            ps = psA.tile([128, SCHUNK], f32, tag="psA")
                            for kt in range(KT):
                                nc.tensor.matmul(
                                    ps[:],
                                    wt[:, kt * 128:(kt + 1) * 128],
                                    xt[kt][:, sc * SCHUNK:(sc + 1) * SCHUNK],
                                    start=(kt == 0), stop=(kt == KT - 1),
                                )
                            dst = dst_sb[:, dt * S + sc * SCHUNK: dt * S + (sc + 1) * SCHUNK]
                            rope_evict(ps, dst, sc)

                # ---- phase A2: V projection (natural layout) ---------------
                wv_pool_cm = tc.tile_pool(name="wv_stream", bufs=2)
                wv_pool = wv_pool_cm.__enter__()
                psA2 = psA
                for nc4 in range(4):
                    wvt = wv_pool.tile([128, KT * 512], bf16, tag="wv")
                    nc.sync.dma_start(out=wvt[:], in_=wv[:, nc4 * KT * 512:(nc4 + 1) * KT * 512])
                    for st in range(ST):
                        ps = psA2.tile([128, SCHUNK], f32, tag="psA")
                        for kt in range(KT):
                            nc.tensor.matmul(
                                ps[:],
                                xt[kt][:, st * 128:(st + 1) * 128],
                                wvt[:, kt * 512:(kt + 1) * 512],
                                start=(kt == 0), stop=(kt == KT - 1),
                            )
                        nc.scalar.copy(v_sb[:, st * H + nc4 * 512: st * H + (nc4 + 1) * 512], ps[:])
                wv_pool_cm.__exit__(None, None, None)

            # ---- phase B: attention -----------------------------------------
            wo_pre_cm = tc.tile_pool(name="wo_pre", bufs=1)
            wo_pre_pool = wo_pre_cm.__enter__()
            with tc.tile_pool(name="expS", bufs=2) as expS_pool, \
                 tc.tile_pool(name="attnT", bufs=1) as attnT_pool, \
                 tc.tile_pool(name="tree", bufs=2) as tree_pool, \
                 tc.tile_pool(name="norm", bufs=4) as norm_pool, \
                 tc.tile_pool(name="psS", bufs=2, space="PSUM") as psS, \
                 tc.tile_pool(name="psAV", bufs=3, space="PSUM") as psAV, \
                 tc.tile_pool(name="psDen", bufs=1, space="PSUM") as psDen:

                attnT_sb = attnT_pool.tile([128, NH * S], bf16, tag="attnT")
                av_sb = [attnT_pool.tile([128, S], bf16, tag=f"avsb{i}",
                                         name=f"avsb{i}") for i in range(2)]
                wo0 = wo_pre_pool.tile([128, KT * 128], bf16, tag="wo_pre")
                for c in range(2):
                    nc.sync.dma_start(out=wo0[:, c * 1024:(c + 1) * 1024],
                                      in_=wo[:, c * 1024:(c + 1) * 1024])

                esum_of = {}
                ps_av_of = {}

                t4_of = {}

                def emit_tree_l1(h, e_sb, i):
                    """Level-1 pairwise add E(2i)+E(2i+1); spread through the
                    kt8 loop so the tree isn't one DVE burst at head end."""
                    E = lambda j: e_sb[:, j * S:(j + 1) * S]
                    t4 = tree_pool.tile([128, S], bf16, tag=f"tree4_{i}",
                                        name=f"t4_{h}_{i}")
                    nc.vector.tensor_add(t4[:], E(2 * i), E(2 * i + 1))
                    t4_of.setdefault(h, []).append(t4)

                def emit_tree_top(h):
                    t4 = t4_of.pop(h)
                    t2 = [tree_pool.tile([128, S], bf16, tag=f"tree2_{i}",
                                         name=f"t2_{h}_{i}") for i in range(2)]
                    for i in range(2):
                        nc.vector.tensor_add(t2[i][:], t4[2 * i][:], t4[2 * i + 1][:])
                    esum = tree_pool.tile([128, S], bf16, tag="esum",
                                          name=f"esum_{h}")
                    nc.vector.tensor_add(esum[:], t2[0][:], t2[1][:])
                    esum_of[h] = esum

                def emit_den_norm(h, qc):
                    """den broadcast (PE) + reciprocal + normalize (DVE).
                    When pipelined, av was staged to SBUF by a DVE copy."""
                    esum = esum_of[h]
                    ps_den = psDen.tile([128, SCHUNK], f32, tag="psDen",
                                        name=f"psden_{h}_{qc}")
                    nc.tensor.matmul(
                        ps_den[:],
                        ones_mat[:],
                        esum[:, qc * SCHUNK:(qc + 1) * SCHUNK],
                        start=True, stop=True,
                    )
                    recipb = norm_pool.tile([128, SCHUNK], f32, tag="recipb",
                                            name=f"recipb_{h}_{qc}")
                    nc.vector.reciprocal_approx_fast(out=recipb[:], in_=ps_den[:])
                    src = av_sb[h % 2][:, qc * SCHUNK:(qc + 1) * SCHUNK] \
                        if PIPELINE_DEN else ps_av_of[h][qc][:]
                    nc.vector.tensor_mul(
                        attnT_sb[:, h * S + qc * SCHUNK: h * S + (qc + 1) * SCHUNK],
                        src, recipb[:])
                    if qc == NSC - 1:
                        del esum_of[h]
                        del ps_av_of[h]

                for h in range(NH):
                    expS_sb = expS_pool.tile([128, ST * S], bf16, tag="expS",
                                             name=f"expS_{h}")
                    ps_av_of[h] = [psAV.tile([128, SCHUNK], f32, tag="psAV",
                                             name=f"psav_{h}_{qc}") for qc in range(NSC)]
                    ps_av = ps_av_of[h]
                    for kt8 in range(ST):
                        ps_s = psS.tile([128, S], f32, tag="psS",
                                        name=f"pss_{h}_{kt8}")
                        for qc in range(NSC):
                            nc.tensor.matmul(
                                ps_s[:, qc * SCHUNK:(qc + 1) * SCHUNK],
                                kT_sb[:, h * S + kt8 * 128: h * S + (kt8 + 1) * 128],
                                qT_sb[:, h * S + qc * SCHUNK: h * S + (qc + 1) * SCHUNK],
                                start=True, stop=True,
                            )
                        e_ap = expS_sb[:, kt8 * S:(kt8 + 1) * S]
                        nc.scalar.activation(
                            e_ap, ps_s[:],
                            func=mybir.ActivationFunctionType.Exp,
                            scale=SCALE,
                        )
                        # spread tree level-1 adds through the loop (first 3)
                        if kt8 in (1, 3, 5):
                            emit_tree_l1(h, expS_sb, kt8 // 2)
                        # pipelined denominator of the previous head, placed
                        # mid-head so its tree/reciprocal chain is ready and
                        # the PE never stalls on it
                        if PIPELINE_DEN and h > 0 and kt8 in (4, 5):
                            emit_den_norm(h - 1, kt8 - 4)
                        for qc in range(NSC):
                            nc.tensor.matmul(
                                ps_av[qc][:],
                                v_sb[:, kt8 * H + h * 128: kt8 * H + (h + 1) * 128],
                                expS_sb[:, kt8 * S + qc * SCHUNK: kt8 * S + (qc + 1) * SCHUNK],
                                start=(kt8 == 0), stop=(kt8 == ST - 1),
                            )
                    if PIPELINE_DEN:
                        # stage AV psum to SBUF on DVE first (frees the psum
                        # pair before the next head needs banks), then finish
                        # the tree
                        for qc in range(NSC):
                            nc.vector.tensor_copy(
                                av_sb[h % 2][:, qc * SCHUNK:(qc + 1) * SCHUNK],
                                ps_av[qc][:])
                        emit_tree_l1(h, expS_sb, 3)
                        emit_tree_top(h)
                    else:
                        emit_tree_l1(h, expS_sb, 3)
                        emit_tree_top(h)
                        for qc in range(NSC):
                            emit_den_norm(h, qc)

                if PIPELINE_DEN:
                    for qc in range(NSC):
                        emit_den_norm(NH - 1, qc)

            # ---- phase C: O-projection -------------------------------------
            with tc.tile_pool(name="wo_stream", bufs=4) as wo_pool, \
                 tc.tile_pool(name="ostage", bufs=2) as ostage_pool, \
                 tc.tile_pool(name="psO", bufs=2, space="PSUM") as psO:
                for ot in range(DT):
                    if ot == 0:
                        wot = wo0
                    else:
                        wot = wo_pool.tile([128, KT * 128], bf16, tag="wo",
                                           name=f"wo_{ot}")
                        nc.sync.dma_start(out=wot[:], in_=wo[:, ot * H:(ot + 1) * H])
                    ps_o = psO.tile([128, S], f32, tag="psO")
                    for dt in range(DT):
                        for qc in range(NSC):
                            nc.tensor.matmul(
                                ps_o[:, qc * SCHUNK:(qc + 1) * SCHUNK],
                                wot[:, dt * 128:(dt + 1) * 128],
                                attnT_sb[:, dt * S + qc * SCHUNK: dt * S + (qc + 1) * SCHUNK],
                                start=(dt == 0), stop=(dt == DT - 1),
                            )
                    o_sb = ostage_pool.tile([128, S], f32, tag="ostage")
                    # per-chunk evict+DMA so the final chunk's copy overlaps
                    # the previous chunk's store (shorter kernel tail)
                    for c in range(2):
                        nc.scalar.copy(o_sb[:, c * SCHUNK:(c + 1) * SCHUNK],
                                       ps_o[:, c * SCHUNK:(c + 1) * SCHUNK])
                        nc.sync.dma_start(
                            out=out[ot * 128:(ot + 1) * 128, c * SCHUNK:(c + 1) * SCHUNK],
                            in_=o_sb[:, c * SCHUNK:(c + 1) * SCHUNK])
            wo_pre_cm.__exit__(None, None, None)
    nc.finalize()
    return nc


def _prep_core_inputs(x_bc, wq_t, wk_t, wv_t, wo_t, cosT_p, sinTs_p):
    # x_bc: (S, H) f32 -> xT partition-major [128, KT*S] bf16
    xT_p = np.ascontiguousarray(
        x_bc.T.reshape(KT, 128, S).transpose(1, 0, 2).reshape(128, KT * S)
    ).astype(BF)
    return {
        "xT": xT_p, "wq": wq_t, "wk": wk_t, "wv": wv_t, "wo": wo_t,
        "cosT": cosT_p, "sinTs": sinTs_p,
    }


def _prep_shared(cos, sin, w_qkv, w_o):
    def dtile_major(w):  # (H, 2048) -> [128, DT*H], lhsT tiles (dt, kt)
        return np.ascontiguousarray(
            w.reshape(KT, 128, DT, 128).transpose(1, 2, 0, 3).reshape(128, DT * H)
        ).astype(BF)

    wq_t = dtile_major(w_qkv[:, :H])
    wk_t = dtile_major(w_qkv[:, H:2 * H])
    wo_t = dtile_major(w_o)
    wv_t = np.ascontiguousarray(
        w_qkv[:, 2 * H:].reshape(KT, 128, 4, 512).transpose(1, 2, 0, 3)
        .reshape(128, 4 * KT * 512)
    ).astype(BF)

    cos_p = np.ones((S, HD), np.float32)
    cos_p[:NENC] = cos
    sin_p = np.zeros((S, HD), np.float32)
    sin_p[:NENC] = sin
    cosT_p = np.ascontiguousarray(cos_p.T)
    sinT = sin_p.T.copy()
    sinTs_p = np.concatenate([-sinT[:64], sinT[64:]], axis=0)
    sinTs_p = np.ascontiguousarray(sinTs_p)
    return wq_t, wk_t, wv_t, wo_t, cosT_p, sinTs_p


_CACHED_NC = None


def kernel(hidden_states, cos, sin, w_qkv, w_o):
    global _CACHED_NC
    from concourse.bass_utils import run_bass_kernel_spmd

    hidden_states = np.asarray(hidden_states, dtype=np.float32)
    cos = np.asarray(cos, dtype=np.float32)
    sin = np.asarray(sin, dtype=np.float32)
    w_qkv = np.asarray(w_qkv, dtype=np.float32)
    w_o = np.asarray(w_o, dtype=np.float32)

    shared = _prep_shared(cos, sin, w_qkv, w_o)
    xs = hidden_states.reshape(B * C, S, H)
    in_maps = [_prep_core_inputs(xs[i], *shared) for i in range(NCORES)]

    if _CACHED_NC is None:
        _CACHED_NC = build_nc()
    res = run_bass_kernel_spmd(_CACHED_NC, in_maps, list(range(NCORES)))

    out_full = np.empty((B * C, S, H), np.float32)
    for i in range(NCORES):
        out_full[i] = np.asarray(res.results[i]["out"], dtype=np.float32).T
    return out_full.reshape(B, C, S, H)
